# revision 4
# baseline (speedup 1.0000x reference)
"""DenseGATConv Trainium2 kernel (8 NeuronCores, SPMD, column-sharded).

Math
----
reference:
    adj = adj with unit diagonal       (binary 0/1)
    h = x @ W
    a_src = h @ att_src ; a_dst = h @ att_dst
    alpha = adj * exp(leaky_relu(a_src[:,None] + a_dst[None,:], 0.2))
    alpha = alpha / alpha.sum(axis=0)
    out = (adj * alpha).T @ h + bias     (adj binary => adj*alpha == alpha)

Key identities used on device:
    leaky_relu(z) = 0.2 z + 0.8 relu(z)
 => exp(lrelu(z)) = exp(0.2 a_src) * exp(0.2 a_dst) * exp(0.8 relu(z))
    exp(0.8 relu(z)) = max(exp(0.8 z), 1) = max(exp(0.8 a_src) exp(0.8 a_dst), 1)
The exp(0.2 a_dst[j]) factor is a per-column constant and cancels between the
numerator and the column-sum, so with
    u_i = exp(0.2 a_src_i), q_j = exp(0.8 a_dst_j)
    M[i,j] = adj[i,j] * max(exp(a_src_i) q_j, u_i)
we get  out[j,:] = (M^T h)[j,:] / colsum(M)[j] + bias.

Sharding: each of 8 cores owns 1024 destination columns j (a column block of
adj). Column sums are then fully local: no cross-core communication.

Per-core device program:
    h      : 64 PSUM-accumulated matmuls from (host-transposed) xT tiles and W
    a_src  : per-tile free-dim reduction of h * att_src (scalar_tensor_tensor
             accum_out) giving per-partition columns; exp'd in chunks
    a_dst  : w_dst = reduce(W * att_dst) on device, then w_dst^T @ xT_local,
             broadcast across partitions with a ones-matmul, exp -> q_rep
    mask   : t2 = max(q_rep * exp(a_src_i), u_i)  (one tensor_scalar op)
             M  = t2 * adj_tile                   (one tensor_tensor op)
    out    : numT[c,j] += h_t^T @ M   ; den[j] += ones^T @ M   (PSUM accum)
Host divides numT by den, transposes, adds bias.

build_nc(reps=R) wraps the whole body in a device-side For_i loop so a single
dispatch runs the kernel R times; test.py uses the R=1 vs R=big wall-clock
slope to estimate HW kernel time (no profiling hooks in this environment).
"""

import numpy as np
from contextlib import ExitStack

import concourse.bass as bass
import concourse.bacc as bacc
import concourse.tile as tile
from concourse import mybir
from concourse.bass_utils import run_bass_kernel_spmd

F32 = mybir.dt.float32
ALU = mybir.AluOpType
ACTF = mybir.ActivationFunctionType

N, C_IN, C_OUT = 8192, 256, 128
NCORES = 8
JB = N // NCORES          # 1024 destination columns per core
NT = N // 128             # 64 i-tiles
CHUNK = 4                 # i-tiles per adj DMA (512KB f32 per DMA)
NCH = NT // CHUNK
GRP = 16                  # a_src exp-chunk size (i-tiles)

_nc_cache = {}


def _emit_body(tc, nc, ctx, tensors):
    (xT_in, xTloc_in, adj_in, W_in, asrc_rep_in, adst_rep_in,
     numT_out, den_out) = tensors

    adj_r = adj_in.rearrange("(c a p) j -> c p a j", a=CHUNK, p=128)

    const = ctx.enter_context(tc.tile_pool(name="const", bufs=1))
    xt_pool = ctx.enter_context(tc.tile_pool(name="xt", bufs=6))
    h_pool = ctx.enter_context(tc.tile_pool(name="h", bufs=NT))
    scratch = ctx.enter_context(tc.tile_pool(name="scr", bufs=2))
    adj_pool = ctx.enter_context(tc.tile_pool(name="adj", bufs=3))
    t2_pool = ctx.enter_context(tc.tile_pool(name="t2", bufs=4))
    m_pool = ctx.enter_context(tc.tile_pool(name="m", bufs=4))
    ps_h = ctx.enter_context(tc.tile_pool(name="psh", bufs=2, space="PSUM"))
    ps_acc = ctx.enter_context(tc.tile_pool(name="psacc", bufs=1, space="PSUM"))

    # ---- constants -------------------------------------------------
    W_sb = const.tile([128, C_IN], F32, name="W_sb")
    nc.sync.dma_start(W_sb[:, 0:128], W_in[:, 0:128])
    nc.sync.dma_start(W_sb[:, 128:256], W_in[:, 128:256])
    attsrc = const.tile([128, C_OUT], F32, name="attsrc")
    nc.sync.dma_start(attsrc[:], asrc_rep_in[:])
    attdst = const.tile([128, C_OUT], F32, name="attdst")
    nc.sync.dma_start(attdst[:], adst_rep_in[:])
    ones_row = const.tile([1, 128], F32, name="ones_row")
    nc.vector.memset(ones_row[:], 1.0)
    ones_col = const.tile([128, 1], F32, name="ones_col")
    nc.vector.memset(ones_col[:], 1.0)

    # ---- a_dst path -> q_rep --------------------------------------
    # w_dst[k] = sum_c W[k, c] att_dst[c]   (free-dim reduce on W tiles)
    wdst = const.tile([128, 2], F32, name="wdst")
    for k in range(2):
        sc = scratch.tile([128, C_OUT], F32, tag="scr", name=f"scw{k}")
        nc.vector.scalar_tensor_tensor(
            sc[:], W_sb[:, k * 128:(k + 1) * 128], 1.0, attdst[:],
            op0=ALU.mult, op1=ALU.mult, accum_out=wdst[:, k:k + 1],
        )
    # a_dst row over the local block: psum[1, 512] x2 = wdst_k^T @ xTloc
    adst_row = const.tile([1, JB], F32, name="adst_row")
    with tc.tile_pool(name="pspre", bufs=1, space="PSUM") as ps_pre:
        for jb in range(JB // 512):
            ap = ps_pre.tile([1, 512], F32, tag="adst", name=f"adstp{jb}")
            for k in range(2):
                xl = xt_pool.tile([128, 512], F32, tag="xtloc",
                                  name=f"xl{jb}_{k}")
                nc.sync.dma_start(
                    xl[:], xTloc_in[k * 128:(k + 1) * 128,
                                    jb * 512:(jb + 1) * 512])
                nc.tensor.matmul(ap[:], lhsT=wdst[:, k:k + 1], rhs=xl[:],
                                 start=(k == 0), stop=(k == 1))
            nc.scalar.copy(adst_row[0:1, jb * 512:(jb + 1) * 512], ap[:])
        # broadcast across partitions and exponentiate:
        # q_rep[p, j] = exp(0.8 * a_dst[j])
        q_rep = const.tile([128, JB], F32, name="q_rep")
        for jb in range(JB // 512):
            qp = ps_pre.tile([128, 512], F32, tag="qrep", name=f"qp{jb}")
            nc.tensor.matmul(
                qp[:], lhsT=ones_row[:],
                rhs=adst_row[0:1, jb * 512:(jb + 1) * 512],
                start=True, stop=True)
            nc.scalar.activation(q_rep[:, jb * 512:(jb + 1) * 512], qp[:],
                                 ACTF.Exp, scale=0.8)

    # ---- h tiles + a_src ------------------------------------------
    h_tiles = []
    asrc_g = [const.tile([128, GRP], F32, tag=f"asrc{g}", name=f"asrc{g}")
              for g in range(NT // GRP)]
    ea_g = [const.tile([128, GRP], F32, tag=f"ea{g}", name=f"ea{g}")
            for g in range(NT // GRP)]   # exp(a_src)
    u_g = [const.tile([128, GRP], F32, tag=f"u{g}", name=f"u{g}")
           for g in range(NT // GRP)]    # exp(0.2 a_src)
    for t in range(NT):
        g, gi = divmod(t, GRP)
        xt0 = xt_pool.tile([128, 128], F32, tag="xt", name=f"xt0_{t}")
        nc.sync.dma_start(xt0[:], xT_in[0:128, t * 128:(t + 1) * 128])
        xt1 = xt_pool.tile([128, 128], F32, tag="xt", name=f"xt1_{t}")
        nc.sync.dma_start(xt1[:], xT_in[128:256, t * 128:(t + 1) * 128])
        hp = ps_h.tile([128, C_OUT], F32, tag="hps", name=f"hps{t}")
        nc.tensor.matmul(hp[:], lhsT=xt0[:], rhs=W_sb[:, 0:128],
                         start=True, stop=False)
        nc.tensor.matmul(hp[:], lhsT=xt1[:], rhs=W_sb[:, 128:256],
                         start=False, stop=True)
        h_t = h_pool.tile([128, C_OUT], F32, tag="h", name=f"h{t}")
        nc.scalar.copy(h_t[:], hp[:])
        h_tiles.append(h_t)
        sc = scratch.tile([128, C_OUT], F32, tag="scr", name=f"sc{t}")
        nc.vector.scalar_tensor_tensor(
            sc[:], h_t[:], 1.0, attsrc[:],
            op0=ALU.mult, op1=ALU.mult,
            accum_out=asrc_g[g][:, gi:gi + 1],
        )
        if gi == GRP - 1:
            nc.scalar.activation(ea_g[g][:], asrc_g[g][:], ACTF.Exp,
                                 scale=1.0)
            nc.scalar.activation(u_g[g][:], asrc_g[g][:], ACTF.Exp,
                                 scale=0.2)

    # ---- main masked-matmul loop ----------------------------------
    num_ps = [ps_acc.tile([C_OUT, 512], F32, tag=f"nps{hf}", name=f"nps{hf}")
              for hf in range(2)]
    den_ps = [ps_acc.tile([1, 512], F32, tag=f"dps{hf}", name=f"dps{hf}")
              for hf in range(2)]
    for c in range(NCH):
        adj_ch = adj_pool.tile([128, CHUNK * JB], F32, tag="adj",
                               name=f"adj{c}")
        nc.sync.dma_start(adj_ch[:], adj_r[c])
        for a in range(CHUNK):
            t = c * CHUNK + a
            g, gi = divmod(t, GRP)
            t2 = t2_pool.tile([128, JB], F32, tag="t2", name=f"t2_{t}")
            nc.vector.tensor_scalar(
                t2[:], q_rep[:], ea_g[g][:, gi:gi + 1], u_g[g][:, gi:gi + 1],
                op0=ALU.mult, op1=ALU.max)
            m = m_pool.tile([128, JB], F32, tag="m", name=f"m{t}")
            nc.vector.tensor_tensor(
                m[:], t2[:], adj_ch[:, a * JB:(a + 1) * JB], op=ALU.mult)
            for hf in range(2):
                ms = m[:, hf * 512:(hf + 1) * 512]
                nc.tensor.matmul(num_ps[hf][:], lhsT=h_tiles[t][:], rhs=ms,
                                 start=(t == 0), stop=(t == NT - 1))
                nc.tensor.matmul(den_ps[hf][:], lhsT=ones_col[:], rhs=ms,
                                 start=(t == 0), stop=(t == NT - 1))

    # ---- epilogue --------------------------------------------------
    num_sb = const.tile([C_OUT, JB], F32, name="num_sb")
    den_sb = const.tile([1, JB], F32, name="den_sb")
    for hf in range(2):
        nc.scalar.copy(num_sb[:, hf * 512:(hf + 1) * 512], num_ps[hf][:])
        nc.scalar.copy(den_sb[0:1, hf * 512:(hf + 1) * 512], den_ps[hf][:])
    nc.sync.dma_start(numT_out[:], num_sb[:])
    nc.sync.dma_start(den_out[:], den_sb[:])


def build_nc(reps=1):
    key = ("nc", reps)
    if key in _nc_cache:
        return _nc_cache[key]
    nc = bacc.Bacc("TRN2", target_bir_lowering=False, debug=False,
                   num_devices=NCORES)

    xT_in = nc.dram_tensor("xT", [C_IN, N], F32, kind="ExternalInput")
    xTloc_in = nc.dram_tensor("xTloc", [C_IN, JB], F32, kind="ExternalInput")
    adj_in = nc.dram_tensor("adjc", [N, JB], F32, kind="ExternalInput")
    W_in = nc.dram_tensor("Wt", [128, C_IN], F32, kind="ExternalInput")
    asrc_rep_in = nc.dram_tensor("attsrc_rep", [128, C_OUT], F32,
                                 kind="ExternalInput")
    adst_rep_in = nc.dram_tensor("attdst_rep", [128, C_OUT], F32,
                                 kind="ExternalInput")

    numT_out = nc.dram_tensor("numT", [C_OUT, JB], F32, kind="ExternalOutput")
    den_out = nc.dram_tensor("den", [1, JB], F32, kind="ExternalOutput")

    tensors = (xT_in, xTloc_in, adj_in, W_in, asrc_rep_in, adst_rep_in,
               numT_out, den_out)

    with tile.TileContext(nc) as tc:
        if reps > 1:
            with tc.For_i(0, reps, 1):
                with ExitStack() as ictx:
                    _emit_body(tc, nc, ictx, tensors)
        else:
            with ExitStack() as ctx:
                _emit_body(tc, nc, ctx, tensors)

    nc.compile()
    _nc_cache[key] = nc
    return nc


def make_in_maps(x, adj, W, att_src, att_dst):
    xT = np.ascontiguousarray(x.T.astype(np.float32, copy=False))
    Wt = np.ascontiguousarray(
        np.concatenate([W[0:128, :], W[128:256, :]], axis=1).astype(np.float32))
    attsrc_rep = np.ascontiguousarray(
        np.broadcast_to(att_src.astype(np.float32), (128, C_OUT)))
    attdst_rep = np.ascontiguousarray(
        np.broadcast_to(att_dst.astype(np.float32), (128, C_OUT)))
    in_maps = []
    for d in range(NCORES):
        adj_d = np.ascontiguousarray(
            adj[:, d * JB:(d + 1) * JB].astype(np.float32, copy=False))
        idx = np.arange(JB)
        adj_d[d * JB + idx, idx] = 1.0          # self loops
        xTloc = np.ascontiguousarray(xT[:, d * JB:(d + 1) * JB])
        in_maps.append({
            "xT": xT, "xTloc": xTloc, "adjc": adj_d, "Wt": Wt,
            "attsrc_rep": attsrc_rep, "attdst_rep": attdst_rep,
        })
    return in_maps


def postprocess(results, bias):
    blocks = []
    for d in range(NCORES):
        numT = results[d]["numT"].astype(np.float64)   # [C_OUT, JB]
        den = results[d]["den"].astype(np.float64)     # [1, JB]
        blocks.append((numT / den).T)
    out = np.concatenate(blocks, axis=0) + bias.astype(np.float64)[None, :]
    return out.astype(np.float32)


def kernel(x, adj, W, att_src, att_dst, bias):
    nc = build_nc()
    in_maps = make_in_maps(x, adj, W, att_src, att_dst)
    res = run_bass_kernel_spmd(nc, in_maps, list(range(NCORES)))
    kernel._last_result = res
    return postprocess(res.results, bias)


# revision 10
# speedup vs baseline: 2.7262x; 2.7262x over previous
"""DenseGATConv Trainium2 kernel (8 NeuronCores, SPMD, column-sharded).

Math
----
reference:
    adj = adj with unit diagonal       (binary 0/1)
    h = x @ W
    a_src = h @ att_src ; a_dst = h @ att_dst
    alpha = adj * exp(leaky_relu(a_src[:,None] + a_dst[None,:], 0.2))
    alpha = alpha / alpha.sum(axis=0)
    out = (adj * alpha).T @ h + bias     (adj binary => adj*alpha == alpha)

Key identities used on device:
    leaky_relu(z) = 0.2 z + 0.8 relu(z)
 => exp(lrelu(z)) = exp(0.2 a_src) * exp(0.2 a_dst) * exp(0.8 relu(z))
    exp(0.8 relu(z)) = max(exp(0.8 z), 1) = max(exp(0.8 a_src) exp(0.8 a_dst), 1)
The exp(0.2 a_dst[j]) factor is a per-column constant and cancels between the
numerator and the column-sum, so with
    u_i = exp(0.2 a_src_i), q_j = exp(0.8 a_dst_j)
    M[i,j] = adj[i,j] * max(exp(a_src_i) q_j, u_i)
we get  out[j,:] = (M^T h)[j,:] / colsum(M)[j] + bias.

Sharding: each of 8 cores owns 1024 destination columns j (a column block of
adj). Column sums are then fully local: no cross-core communication.

Per-core device program:
    h      : 64 PSUM-accumulated matmuls from (host-transposed) xT tiles and W
    a_src  : per-tile free-dim reduction of h * att_src (scalar_tensor_tensor
             accum_out) giving per-partition columns; exp'd in chunks
    a_dst  : w_dst = reduce(W * att_dst) on device, then w_dst^T @ xT_local,
             broadcast across partitions with a ones-matmul, exp -> q_rep
    mask   : t2 = max(q_rep * exp(a_src_i), u_i)  (one tensor_scalar op)
             M  = t2 * adj_tile                   (one tensor_tensor op)
    out    : numT[c,j] += h_t^T @ M   ; den[j] += ones^T @ M   (PSUM accum)
Host divides numT by den, transposes, adds bias.

build_nc(reps=R) wraps the whole body in a device-side For_i loop so a single
dispatch runs the kernel R times; test.py uses the R=1 vs R=big wall-clock
slope to estimate HW kernel time (no profiling hooks in this environment).
"""

import numpy as np
import ml_dtypes
from contextlib import ExitStack

import concourse.bass as bass
import concourse.bacc as bacc
import concourse.tile as tile
from concourse import mybir
from concourse.bass_utils import run_bass_kernel_spmd

F32 = mybir.dt.float32
F32R = mybir.dt.float32r
BF16 = mybir.dt.bfloat16
FP16 = mybir.dt.float16
ALU = mybir.AluOpType
ACTF = mybir.ActivationFunctionType

# --- perf config ---
# fp16 everywhere on the N^2 path: 1 cyc/row PE (vs 4 for fp32), 16-bit DVE
# modes, half DMA -- with 8x the mantissa of bf16. All values fit fp16 range
# (t2 <= ~1100 given the data distributions; x,W,h,adj all small).
ADJ_BF16 = True      # adjacency shipped 16-bit (exact: values are 0/1)
CHAIN_BF16 = True    # 16-bit q_rep/t2/m chain + 16-bit h weights
CH_DT = FP16         # the 16-bit dtype used for the chain
MM_F32R = True       # when not CHAIN_BF16: bitcast num/den matmuls to f32r

N, C_IN, C_OUT = 8192, 256, 128
NCORES = 8
JB = N // NCORES          # 1024 destination columns per core
NT = N // 128             # 64 i-tiles
CHUNK = 4                 # i-tiles per adj DMA (1MB fp16 per DMA)
NCH = NT // CHUNK
GRP = 8                   # a_src exp-chunk size (i-tiles)

_nc_cache = {}


def _emit_body(tc, nc, ctx, tensors):
    (xT_in, xTloc_in, adj_in, W_in, asrc_rep_in, adst_rep_in,
     numT_out, den_out) = tensors

    adj_r = adj_in.rearrange("(c a p) j -> c p a j", a=CHUNK, p=128)

    const = ctx.enter_context(tc.tile_pool(name="const", bufs=1))
    xt_pool = ctx.enter_context(tc.tile_pool(name="xt", bufs=4))
    h_pool = ctx.enter_context(tc.tile_pool(name="h", bufs=NT))
    scratch = ctx.enter_context(tc.tile_pool(name="scr", bufs=2))
    adj_pool = ctx.enter_context(tc.tile_pool(name="adj", bufs=3))
    t2_pool = ctx.enter_context(tc.tile_pool(name="t2", bufs=6))
    m_pool = ctx.enter_context(tc.tile_pool(name="m", bufs=6))
    ps_h = ctx.enter_context(tc.tile_pool(name="psh", bufs=2, space="PSUM"))
    ps_acc = ctx.enter_context(tc.tile_pool(name="psacc", bufs=1, space="PSUM"))

    # ---- constants -------------------------------------------------
    W_sb = const.tile([128, C_IN], CH_DT, name="W_sb")
    nc.sync.dma_start(W_sb[:, 0:128], W_in[:, 0:128])
    nc.sync.dma_start(W_sb[:, 128:256], W_in[:, 128:256])
    attsrc = const.tile([128, C_OUT], F32, name="attsrc")
    nc.sync.dma_start(attsrc[:], asrc_rep_in[:])
    attdst = const.tile([128, C_OUT], F32, name="attdst")
    nc.sync.dma_start(attdst[:], adst_rep_in[:])
    ones_row = const.tile([1, 128], F32, name="ones_row")
    nc.vector.memset(ones_row[:], 1.0)
    ones_col = const.tile([128, 1], F32, name="ones_col")
    nc.vector.memset(ones_col[:], 1.0)
    ones_col_b = const.tile([128, 1], CH_DT, name="ones_col_b")
    nc.vector.memset(ones_col_b[:], 1.0)

    # ---- a_dst path -> q_rep --------------------------------------
    # w_dst[k] = sum_c W[k, c] att_dst[c]   (free-dim reduce on W tiles)
    wdst = const.tile([128, 2], F32, name="wdst")
    for k in range(2):
        sc = scratch.tile([128, C_OUT], F32, tag="scr", name=f"scw{k}")
        nc.vector.scalar_tensor_tensor(
            sc[:], W_sb[:, k * 128:(k + 1) * 128], 1.0, attdst[:],
            op0=ALU.mult, op1=ALU.mult, accum_out=wdst[:, k:k + 1],
        )
    wdst_h = const.tile([128, 2], CH_DT, name="wdst_h")
    nc.vector.tensor_copy(wdst_h[:], wdst[:])
    # a_dst row over the local block: psum[1, 512] x2 = wdst_k^T @ xTloc
    adst_row = const.tile([1, JB], F32, name="adst_row")
    with tc.tile_pool(name="pspre", bufs=1, space="PSUM") as ps_pre:
        for jb in range(JB // 512):
            ap = ps_pre.tile([1, 512], F32, tag="adst", name=f"adstp{jb}")
            for k in range(2):
                xl = xt_pool.tile([128, 512], CH_DT, tag="xtloc",
                                  name=f"xl{jb}_{k}")
                nc.sync.dma_start(
                    xl[:], xTloc_in[k * 128:(k + 1) * 128,
                                    jb * 512:(jb + 1) * 512])
                nc.tensor.matmul(ap[:], lhsT=wdst_h[:, k:k + 1], rhs=xl[:],
                                 start=(k == 0), stop=(k == 1))
            nc.scalar.copy(adst_row[0:1, jb * 512:(jb + 1) * 512], ap[:])
        # broadcast across partitions and exponentiate:
        # q_rep[p, j] = exp(0.8 * a_dst[j])
        q_rep = const.tile([128, JB], CH_DT if CHAIN_BF16 else F32,
                           name="q_rep")
        for jb in range(JB // 512):
            qp = ps_pre.tile([128, 512], F32, tag="qrep", name=f"qp{jb}")
            nc.tensor.matmul(
                qp[:], lhsT=ones_row[:],
                rhs=adst_row[0:1, jb * 512:(jb + 1) * 512],
                start=True, stop=True)
            nc.scalar.activation(q_rep[:, jb * 512:(jb + 1) * 512], qp[:],
                                 ACTF.Exp, scale=0.8)

    # ---- h tiles + a_src ------------------------------------------
    # xT loaded in [128, 2048] chunks (1 MB DMAs) per k-block
    h_tiles = []
    hb_tiles = []
    asrc_g = [const.tile([128, GRP], F32, tag=f"asrc{g}", name=f"asrc{g}")
              for g in range(NT // GRP)]
    ea_g = [const.tile([128, GRP], F32, tag=f"ea{g}", name=f"ea{g}")
            for g in range(NT // GRP)]   # exp(a_src)
    u_g = [const.tile([128, GRP], F32, tag=f"u{g}", name=f"u{g}")
           for g in range(NT // GRP)]    # exp(0.2 a_src)
    XB = 16   # i-tiles per xT chunk
    for cx in range(NT // XB):
        xc = [xt_pool.tile([128, XB * 128], CH_DT, tag="xtc",
                           name=f"xc{cx}_{k}")
              for k in range(2)]
        for k in range(2):
            nc.sync.dma_start(
                xc[k][:], xT_in[k * 128:(k + 1) * 128,
                                cx * XB * 128:(cx + 1) * XB * 128])
        for ti in range(XB):
            t = cx * XB + ti
            g, gi = divmod(t, GRP)
            hp = ps_h.tile([128, C_OUT], F32, tag="hps", name=f"hps{t}")
            nc.tensor.matmul(hp[:], lhsT=xc[0][:, ti * 128:(ti + 1) * 128],
                             rhs=W_sb[:, 0:128], start=True, stop=False)
            nc.tensor.matmul(hp[:], lhsT=xc[1][:, ti * 128:(ti + 1) * 128],
                             rhs=W_sb[:, 128:256], start=False, stop=True)
            h_t = h_pool.tile([128, C_OUT],
                              CH_DT if CHAIN_BF16 else F32,
                              tag="h", name=f"h{t}")
            nc.scalar.copy(h_t[:], hp[:])
            h_tiles.append(h_t)
            hb_tiles.append(h_t)
            sc = scratch.tile([128, C_OUT], F32, tag="scr", name=f"sc{t}")
            nc.vector.scalar_tensor_tensor(
                sc[:], h_t[:], 1.0, attsrc[:],
                op0=ALU.mult, op1=ALU.mult,
                accum_out=asrc_g[g][:, gi:gi + 1],
            )
            if gi == GRP - 1:
                nc.scalar.activation(ea_g[g][:], asrc_g[g][:], ACTF.Exp,
                                     scale=1.0)
                nc.scalar.activation(u_g[g][:], asrc_g[g][:], ACTF.Exp,
                                     scale=0.2)

    # ---- main masked-matmul loop ----------------------------------
    num_ps = [ps_acc.tile([C_OUT, 512], F32, tag=f"nps{hf}", name=f"nps{hf}")
              for hf in range(2)]
    den_ps = [ps_acc.tile([1, 512], F32, tag=f"dps{hf}", name=f"dps{hf}")
              for hf in range(2)]
    chain_dt = CH_DT if CHAIN_BF16 else F32
    adj_dt = CH_DT if ADJ_BF16 else F32
    for c in range(NCH):
        adj_ch = adj_pool.tile([128, CHUNK * JB], adj_dt, tag="adj",
                               name=f"adj{c}")
        nc.sync.dma_start(adj_ch[:], adj_r[c])
        for a in range(CHUNK):
            t = c * CHUNK + a
            g, gi = divmod(t, GRP)
            t2 = t2_pool.tile([128, JB], chain_dt, tag="t2", name=f"t2_{t}")
            nc.vector.tensor_scalar(
                t2[:], q_rep[:], ea_g[g][:, gi:gi + 1], u_g[g][:, gi:gi + 1],
                op0=ALU.mult, op1=ALU.max)
            m = m_pool.tile([128, JB], chain_dt, tag="m", name=f"m{t}")
            nc.vector.tensor_tensor(
                m[:], t2[:], adj_ch[:, a * JB:(a + 1) * JB], op=ALU.mult)
            if CHAIN_BF16:
                lw_num, lw_den = hb_tiles[t][:], ones_col_b[:]
                mview = m
            elif MM_F32R:
                lw_num = h_tiles[t][:].bitcast(F32R)
                lw_den = ones_col[:].bitcast(F32R)
                mview = m[:].bitcast(F32R)
            else:
                lw_num, lw_den = h_tiles[t][:], ones_col[:]
                mview = m
            for hf in range(2):
                ms = mview[:, hf * 512:(hf + 1) * 512]
                nc.tensor.matmul(num_ps[hf][:], lhsT=lw_num, rhs=ms,
                                 start=(t == 0), stop=(t == NT - 1))
                nc.tensor.matmul(den_ps[hf][:], lhsT=lw_den, rhs=ms,
                                 start=(t == 0), stop=(t == NT - 1))

    # ---- epilogue --------------------------------------------------
    num_sb = const.tile([C_OUT, JB], F32, name="num_sb")
    den_sb = const.tile([1, JB], F32, name="den_sb")
    for hf in range(2):
        nc.scalar.copy(num_sb[:, hf * 512:(hf + 1) * 512], num_ps[hf][:])
        nc.scalar.copy(den_sb[0:1, hf * 512:(hf + 1) * 512], den_ps[hf][:])
    nc.sync.dma_start(numT_out[:], num_sb[:])
    nc.sync.dma_start(den_out[:], den_sb[:])


def build_nc(reps=1):
    key = ("nc", reps)
    if key in _nc_cache:
        return _nc_cache[key]
    nc = bacc.Bacc("TRN2", target_bir_lowering=False, debug=False,
                   num_devices=NCORES)

    xT_in = nc.dram_tensor("xT", [C_IN, N], CH_DT, kind="ExternalInput")
    xTloc_in = nc.dram_tensor("xTloc", [C_IN, JB], CH_DT,
                              kind="ExternalInput")
    adj_in = nc.dram_tensor("adjc", [N, JB],
                            CH_DT if ADJ_BF16 else F32,
                            kind="ExternalInput")
    W_in = nc.dram_tensor("Wt", [128, C_IN], CH_DT, kind="ExternalInput")
    asrc_rep_in = nc.dram_tensor("attsrc_rep", [128, C_OUT], F32,
                                 kind="ExternalInput")
    adst_rep_in = nc.dram_tensor("attdst_rep", [128, C_OUT], F32,
                                 kind="ExternalInput")

    numT_out = nc.dram_tensor("numT", [C_OUT, JB], F32, kind="ExternalOutput")
    den_out = nc.dram_tensor("den", [1, JB], F32, kind="ExternalOutput")

    tensors = (xT_in, xTloc_in, adj_in, W_in, asrc_rep_in, adst_rep_in,
               numT_out, den_out)

    with tile.TileContext(nc) as tc:
        if reps > 1:
            with tc.For_i(0, reps, 1):
                with ExitStack() as ictx:
                    _emit_body(tc, nc, ictx, tensors)
        else:
            with ExitStack() as ctx:
                _emit_body(tc, nc, ctx, tensors)

    nc.compile()
    _nc_cache[key] = nc
    return nc


def make_in_maps(x, adj, W, att_src, att_dst):
    ch_np = np.float16 if CH_DT == FP16 else ml_dtypes.bfloat16
    xT = np.ascontiguousarray(x.T.astype(np.float32, copy=False)).astype(ch_np)
    Wt = np.ascontiguousarray(
        np.concatenate([W[0:128, :], W[128:256, :]], axis=1)).astype(ch_np)
    attsrc_rep = np.ascontiguousarray(
        np.broadcast_to(att_src.astype(np.float32), (128, C_OUT)))
    attdst_rep = np.ascontiguousarray(
        np.broadcast_to(att_dst.astype(np.float32), (128, C_OUT)))
    in_maps = []
    for d in range(NCORES):
        adj_d = np.ascontiguousarray(
            adj[:, d * JB:(d + 1) * JB].astype(np.float32, copy=False))
        idx = np.arange(JB)
        adj_d[d * JB + idx, idx] = 1.0          # self loops
        if ADJ_BF16:
            adj_d = adj_d.astype(ch_np)                # 0/1: exact
        xTloc = np.ascontiguousarray(xT[:, d * JB:(d + 1) * JB])
        in_maps.append({
            "xT": xT, "xTloc": xTloc, "adjc": adj_d, "Wt": Wt,
            "attsrc_rep": attsrc_rep, "attdst_rep": attdst_rep,
        })
    return in_maps


def postprocess(results, bias):
    blocks = []
    for d in range(NCORES):
        numT = results[d]["numT"].astype(np.float64)   # [C_OUT, JB]
        den = results[d]["den"].astype(np.float64)     # [1, JB]
        blocks.append((numT / den).T)
    out = np.concatenate(blocks, axis=0) + bias.astype(np.float64)[None, :]
    return out.astype(np.float32)


def kernel(x, adj, W, att_src, att_dst, bias):
    nc = build_nc()
    in_maps = make_in_maps(x, adj, W, att_src, att_dst)
    res = run_bass_kernel_spmd(nc, in_maps, list(range(NCORES)))
    kernel._last_result = res
    return postprocess(res.results, bias)
